# revision 27
# baseline (speedup 1.0000x reference)
"""Trainium2 Bass kernel for nn_AttentionTanh (B=8, S=2048, F=1024, U=256).

Data-parallel over batch: each of the 8 NeuronCores computes the full
attention for one batch example. No collectives.

Per-core dataflow (all matmuls via TensorE, out = lhsT.T @ rhs):
  xT   [F, S]  (host-swizzled bf16 input shard, F on partitions)
  QT   [u, s] = tanh(Wq.T @ x.T)  -> matmul(lhsT=Wq[f,u], rhs=xT[f,s])
  KT   [u, s] = tanh(Wk.T @ x.T)      QT/KT stored fp8e4 (scores run in
                                      fp8 DoubleRow; tanh bounds |q|<=1)
  V    [s, u] = tanh(x @ Wv)      -> matmul(lhsT=xT[f,s], rhs=Wv[f,u])
                V gets two fused ones-columns so the out-matmul also
                produces the softmax denominator (cols U:U+2).
  eST  [t, q] = exp(scale * K.T q) -> ONE fp8 DoubleRow matmul per
                (t-tile, q-block): contracts the full U=256 across the
                two uo planes of kT/qT at 2 rows/cycle.
                (tanh bounds scores to [-8, 8]; no max subtraction)
  out  [q, u] = (eST.T @ [V | 1 1]) row-normalized by column U (bf16).

Inputs are cast to bf16 on the host: halves HBM traffic (x: 8MB->4MB
per core) and the projection matmuls get Fast Weight Load.
"""

import os
import sys

import numpy as np
import ml_dtypes

for _p in ("/opt/trn_rl_repo", "/root/.axon_site/_ro/trn_rl_repo"):
    if os.path.isdir(_p) and _p not in sys.path:
        sys.path.append(_p)

import concourse.bass as bass
import concourse.mybir as mybir
import concourse.tile as tile
from concourse.bass_utils import run_bass_kernel_spmd

P = 128
B, S, F, U = 8, 2048, 1024, 256
FO, SO, UO = F // P, S // P, U // P  # 8, 16, 2
SB = 512                             # s-block width for DMA/projections
NSB = S // SB                        # 4
QB = 512                             # query-block width (free dim of eST)
NQB = S // QB                        # 4
SCALE = 1.0 / float(np.sqrt(F))      # 1/32
VW = U + 2                           # V plus fused ones columns
F32 = mybir.dt.float32
BF16 = mybir.dt.bfloat16
FP8 = mybir.dt.float8e4
DR = mybir.MatmulPerfMode.DoubleRow

NP_BF16 = ml_dtypes.bfloat16


def _split_matmul_waits(nc):
    """Walrus instruction structs have a single sem-wait slot (EventSemaphore
    has two). Peel excess waits onto NoOps (plain wait instructions on the
    same engine) inserted just before the overloaded instruction."""
    n = 0
    for bb in nc.m.functions[0].blocks:
        new_insts = []
        for inst in bb.instructions:
            cap = 2 if isinstance(inst, mybir.InstEventSemaphore) else 1
            if (
                inst.sync_info
                and inst.sync_info.on_wait
                and len(inst.sync_info.on_wait) > cap
            ):
                waits = list(inst.sync_info.on_wait)
                for w in waits[cap:]:
                    n += 1
                    nop = mybir.InstNoOp(name=f"I-xwait-{n}", ins=[], outs=[])
                    nop.engine = inst.engine
                    nop.sync_info = mybir.SyncInfo(on_wait=[w], on_update=[])
                    new_insts.append(nop)
                inst.sync_info.on_wait = waits[:cap]
            new_insts.append(inst)
        bb.instructions[:] = new_insts
    return n


def build_nc(qk_fp8=True, split_waits=True):
    qk_dt = FP8 if qk_fp8 else BF16

    nc = bass.Bass()
    # Host pre-swizzles inputs to SBUF-matching layouts so every DMA is one
    # long contiguous run per partition (8KB for x blocks, 2-4KB for weights).
    xT_d = nc.declare_dram_parameter("xT", [P, NSB, FO, SB], BF16, isOutput=False)
    w_d = {
        k: nc.declare_dram_parameter(k, [P, UO, FO, P], BF16, isOutput=False)
        for k in ("Wq", "Wk")
    }
    w_d["Wv"] = nc.declare_dram_parameter("Wv", [P, FO, U], BF16, isOutput=False)
    out_d = nc.declare_dram_parameter("out", [S, U], BF16, isOutput=True)

    TANH = mybir.ActivationFunctionType.Tanh
    EXP = mybir.ActivationFunctionType.Exp

    with tile.TileContext(nc) as tc:
        with (
            tc.tile_pool(name="wpool", bufs=1) as wpool,
            tc.tile_pool(name="qkv", bufs=1) as qkv,
            tc.tile_pool(name="smalls", bufs=1) as smalls,
            tc.tile_pool(name="recs", bufs=2) as recs,
            tc.tile_pool(name="evac", bufs=6) as evac,
            tc.tile_pool(name="exps", bufs=4) as exps,
            tc.tile_pool(name="ps_big", bufs=3, space="PSUM") as ps_big,
            tc.tile_pool(name="ps_v", bufs=2, space="PSUM") as ps_v,
            tc.tile_pool(name="ps_o", bufs=3, space="PSUM") as ps_o,
        ):
            # ---- phase 1: loads + projections. xT lives only here; its
            # SBUF space is released to the exp tiles afterwards. ----
            with tc.tile_pool(name="xpool", bufs=1) as xpool:
                # All DMAs ride the sync/SP queue: SP-issued DMAs fan out
                # over many SDMA engines, while scalar/gpsimd-issued DMAs
                # serialize on one engine (~3x slower — measured). Wq and
                # x-block 0 go first; block 0 is further split per fo chunk
                # so the first QT matmul starts as soon as possible.
                xT = xpool.tile([P, NSB, FO, SB], BF16)
                w_t = {
                    "Wq": wpool.tile([P, UO, FO, P], BF16, tag="Wq", name="w_Wq"),
                    "Wk": wpool.tile([P, UO, FO, P], BF16, tag="Wk", name="w_Wk"),
                    "Wv": wpool.tile([P, FO, U], BF16, tag="Wv", name="w_Wv"),
                }

                def dma_w(k, uo=None):
                    if uo is None:
                        nc.sync.dma_start(w_t[k][:], w_d[k][:])
                    else:
                        nc.sync.dma_start(w_t[k][:, uo], w_d[k][:, uo])

                def dma_x(sb, split=False):
                    if split:
                        for fo in range(FO):
                            nc.sync.dma_start(xT[:, sb, fo, :], xT_d[:, sb, fo, :])
                    else:
                        nc.sync.dma_start(xT[:, sb, :, :], xT_d[:, sb, :, :])

                # Byte-ordered so each consumer's data lands just in time:
                # Wq half 0 + x0-chunk 0 feed the first QT group; Wq half 1
                # right after chunk 0 (QT-uo1 re-reads resident chunks, so it
                # must not queue behind the whole x0 stream); Wk halves before
                # KT of block 0; Wv before V of block 0.
                dma_w("Wq", 0)
                for fo in range(FO):
                    nc.sync.dma_start(xT[:, 0, fo, :], xT_d[:, 0, fo, :])
                    if fo == 0:
                        dma_w("Wq", 1)
                dma_w("Wk", 0)
                dma_w("Wk", 1)
                dma_w("Wv")
                for sb in range(1, NSB):
                    dma_x(sb)

                # PE warmup: junk matmuls on a zeroed tile keep the PE busy
                # while the x DMAs land, so HAM un-throttles before real work.
                # gpsimd memsets the tile (bf16 bit pattern packed directly):
                # it is free ~1.5us before Vector at startup.
                warm = smalls.tile([P, SB], BF16, tag="warm")
                nc.gpsimd.memset(warm[:], 0.0)
                ps_w = ps_v.tile([P, SB], F32, tag="ps_v", name="ps_w")
                for _ in range(11):
                    nc.tensor.matmul(
                        ps_w[:], warm[:, :P], warm[:], start=True, stop=True
                    )

                # ---- projections (per s-block so PE starts as DMA lands) ----
                qT = qkv.tile([P, UO, S], qk_dt, tag="qT")
                kT = qkv.tile([P, UO, S], qk_dt, tag="kT")
                vv = qkv.tile([P, SO, VW], BF16, tag="vv")
                nc.gpsimd.memset(vv[:, :, U:VW], 1.0)

                ex_tiles = [None] * NQB
                for sb in range(NSB):
                    sl = slice(sb * SB, (sb + 1) * SB)
                    for wname, dst in (("Wq", qT), ("Wk", kT)):
                        for uo in range(UO):
                            ps = ps_big.tile([P, SB], F32, tag="ps_big")
                            for fo in range(FO):
                                nc.tensor.matmul(
                                    ps[:],
                                    w_t[wname][:, uo, fo, :],
                                    xT[:, sb, fo, :],
                                    start=(fo == 0),
                                    stop=(fo == FO - 1),
                                )
                                if sb == 0 and wname == "Wq" and uo == 0 and fo:
                                    # junk filler between the chunk-paced
                                    # first group's matmuls: absorbs x0 DMA
                                    # jitter without idling the PE (an idle
                                    # PE also loses its clock ramp)
                                    nc.tensor.matmul(
                                        ps_w[:, :U],
                                        warm[:, :P],
                                        warm[:, :U],
                                        start=True,
                                        stop=True,
                                    )
                            nc.scalar.activation(dst[:, uo, sl], ps[:], TANH)
                    for so in range(sb * SB // P, (sb + 1) * SB // P):
                        si = (so % (SB // P)) * P
                        ps = ps_v.tile([P, U], F32, tag="ps_v")
                        for fo in range(FO):
                            nc.tensor.matmul(
                                ps[:],
                                xT[:, sb, fo, si : si + P],
                                w_t["Wv"][:, fo, :],
                                start=(fo == 0),
                                stop=(fo == FO - 1),
                            )
                        nc.scalar.activation(vv[:, so, :U], ps[:], TANH)
                    # scores for the t-chunks this block's K just produced:
                    # fills PE gaps while the next x block's DMA lands. The
                    # needed qT q-slices come from blocks <= sb, available
                    # for qb <= sb; later qb wait for their qT (handled by
                    # Tile deps, but emitted only when ready to avoid stalls).
                    for qb in range(NQB):
                        if ex_tiles[qb] is None:
                            ex_tiles[qb] = exps.tile(
                                [P, SO, QB], BF16, tag="ex", name=f"ex{qb}"
                            )
                        if qb > sb:
                            continue
                        if sb == NSB - 1 and qb >= 2:
                            # deferred to phase 2: the last block otherwise
                            # ends with 16 back-to-back exps that swamp the
                            # Scalar engine right when the out-phase starts
                            continue
                        qsl = slice(qb * QB, (qb + 1) * QB)
                        for to in range(sb * (SO // NSB), (sb + 1) * (SO // NSB)):
                            ps = ps_big.tile([P, QB], F32, tag="ps_big")
                            if qk_fp8:
                                nc.tensor.matmul(
                                    ps[:],
                                    kT[:, :, to * P : (to + 1) * P],
                                    qT[:, :, qsl],
                                    start=True,
                                    stop=True,
                                    perf_mode=DR,
                                )
                            else:
                                for uo in range(UO):
                                    nc.tensor.matmul(
                                        ps[:],
                                        kT[:, uo, to * P : (to + 1) * P],
                                        qT[:, uo, qsl],
                                        start=(uo == 0),
                                        stop=(uo == UO - 1),
                                    )
                            nc.scalar.activation(
                                ex_tiles[qb][:, to, :], ps[:], EXP, scale=SCALE
                            )

            # ---- phase 2: remaining scores + output per query block.
            # Block qb's leftover scores (t-tiles from earlier s-blocks,
            # to < 4*qb) are emitted interleaved into block qb-1's output
            # groups, so their exp evacuations run on the Scalar engine
            # while the PE chews the previous block's out-matmuls. Each
            # out-group accumulates its freshest t-tiles LAST. ----
            def emit_score2(qb, to):
                ps = ps_big.tile([P, QB], F32, tag="ps_big")
                qsl = slice(qb * QB, (qb + 1) * QB)
                if qk_fp8:
                    nc.tensor.matmul(
                        ps[:],
                        kT[:, :, to * P : (to + 1) * P],
                        qT[:, :, qsl],
                        start=True,
                        stop=True,
                        perf_mode=DR,
                    )
                else:
                    for uo in range(UO):
                        nc.tensor.matmul(
                            ps[:],
                            kT[:, uo, to * P : (to + 1) * P],
                            qT[:, uo, qsl],
                            start=(uo == 0),
                            stop=(uo == UO - 1),
                        )
                nc.scalar.activation(
                    ex_tiles[qb][:, to, :], ps[:], EXP, scale=SCALE
                )

            last4 = list(range(SO - SO // NSB, SO))  # sb=3's t-tiles
            pend = {
                1: list(range(1 * (SO // NSB))),
                2: list(range(2 * (SO // NSB))) + last4,
                3: list(range(3 * (SO // NSB))) + last4,
            }
            # inline-available t-tiles lead each out accumulation; pend
            # tiles (exp'd during the previous block's outs) come last,
            # in emission order
            inline = {
                0: list(range(SO)),
                1: list(range(SO // NSB, SO)),
                2: list(range(2 * (SO // NSB), 3 * (SO // NSB))),
                3: [],
            }
            for qb in range(NQB):
                ex = ex_tiles[qb]
                nxt = pend.get(qb + 1, [])
                per = (len(nxt) + (QB // P) - 1) // (QB // P) if nxt else 0
                for ss in range(QB // P):
                    for to in nxt[ss * per : (ss + 1) * per]:
                        emit_score2(qb + 1, to)
                    s0 = qb * QB + ss * P
                    ps = ps_o.tile([P, VW], F32, tag="ps_o")
                    to_order = inline[qb] + pend.get(qb, [])
                    for n, to in enumerate(to_order):
                        nc.tensor.matmul(
                            ps[:],
                            ex[:, to, ss * P : (ss + 1) * P],
                            vv[:, to, :],
                            start=(n == 0),
                            stop=(n == SO - 1),
                        )
                    rec = recs.tile([P, 1], F32, tag="rec")
                    nc.vector.reciprocal(rec[:], ps[:, U : U + 1])
                    ot = evac.tile([P, U], BF16, tag="ot")
                    nc.vector.tensor_scalar_mul(ot[:], ps[:, :U], rec[:])
                    nc.sync.dma_start(out_d[s0 : s0 + P, :], ot[:])

    if split_waits:
        _split_matmul_waits(nc)
    return nc


_NC_CACHE = {}


def _get_nc(key=True):
    if key not in _NC_CACHE:
        _NC_CACHE[key] = build_nc(qk_fp8=key)
    return _NC_CACHE[key]


def _swizzle_w(w):
    # [F, U] -> [fi, fo, u]: contiguous 4KB per partition row.
    w = np.asarray(w, dtype=np.float32)
    return np.ascontiguousarray(
        w.reshape(FO, P, U).transpose(1, 0, 2).astype(NP_BF16)
    )


def _swizzle_w_halves(w):
    # [F, U] -> [fi, uo, fo, ui]: each uo half is one contiguous 2KB run
    # per partition, so it can be DMA'd independently.
    w = np.asarray(w, dtype=np.float32)
    return np.ascontiguousarray(
        w.reshape(FO, P, UO, P).transpose(1, 2, 0, 3).astype(NP_BF16)
    )


def _swizzle_x(xb):
    # [S, F] -> xT [fi, sb, fo, s]: each s-block DMA is one contiguous 8KB
    # run per partition.
    xT = np.asarray(xb, dtype=np.float32).T  # [F, S]
    return np.ascontiguousarray(
        xT.reshape(FO, P, NSB, SB).transpose(1, 2, 0, 3).astype(NP_BF16)
    )


def make_in_maps(x, Wq, Wk, Wv):
    Wq, Wk = _swizzle_w_halves(Wq), _swizzle_w_halves(Wk)
    Wv = _swizzle_w(Wv)
    return [
        {"xT": _swizzle_x(x[b]), "Wq": Wq, "Wk": Wk, "Wv": Wv}
        for b in range(B)
    ]


def kernel(x, Wq, Wk, Wv):
    nc = _get_nc()
    in_maps = make_in_maps(x, Wq, Wk, Wv)
    res = run_bass_kernel_spmd(nc, in_maps, core_ids=list(range(B)))
    return np.stack(
        [np.asarray(res.results[i]["out"], dtype=np.float32) for i in range(B)],
        axis=0,
    )


# revision 29
# speedup vs baseline: 1.0051x; 1.0051x over previous
"""Trainium2 Bass kernel for nn_AttentionTanh (B=8, S=2048, F=1024, U=256).

Data-parallel over batch: each of the 8 NeuronCores computes the full
attention for one batch example. No collectives.

Per-core dataflow (all matmuls via TensorE, out = lhsT.T @ rhs):
  xT   [F, S]  (host-swizzled bf16 input shard, F on partitions)
  QT   [u, s] = tanh(Wq.T @ x.T)  -> matmul(lhsT=Wq[f,u], rhs=xT[f,s])
  KT   [u, s] = tanh(Wk.T @ x.T)      QT/KT stored fp8e4 (scores run in
                                      fp8 DoubleRow; tanh bounds |q|<=1)
  V    [s, u] = tanh(x @ Wv)      -> matmul(lhsT=xT[f,s], rhs=Wv[f,u])
                V gets two fused ones-columns so the out-matmul also
                produces the softmax denominator (cols U:U+2).
  eST  [t, q] = exp(scale * K.T q) -> ONE fp8 DoubleRow matmul per
                (t-tile, q-block): contracts the full U=256 across the
                two uo planes of kT/qT at 2 rows/cycle.
                (tanh bounds scores to [-8, 8]; no max subtraction)
  out  [q, u] = (eST.T @ [V | 1 1]) row-normalized by column U (bf16).

Inputs are cast to bf16 on the host: halves HBM traffic (x: 8MB->4MB
per core) and the projection matmuls get Fast Weight Load.
"""

import os
import sys

import numpy as np
import ml_dtypes

for _p in ("/opt/trn_rl_repo", "/root/.axon_site/_ro/trn_rl_repo"):
    if os.path.isdir(_p) and _p not in sys.path:
        sys.path.append(_p)

import concourse.bass as bass
import concourse.mybir as mybir
import concourse.tile as tile
from concourse.bass_utils import run_bass_kernel_spmd

P = 128
B, S, F, U = 8, 2048, 1024, 256
FO, SO, UO = F // P, S // P, U // P  # 8, 16, 2
SB = 512                             # s-block width for DMA/projections
NSB = S // SB                        # 4
QB = 512                             # query-block width (free dim of eST)
NQB = S // QB                        # 4
SCALE = 1.0 / float(np.sqrt(F))      # 1/32
VW = U + 2                           # V plus fused ones columns
F32 = mybir.dt.float32
BF16 = mybir.dt.bfloat16
FP8 = mybir.dt.float8e4
DR = mybir.MatmulPerfMode.DoubleRow

NP_BF16 = ml_dtypes.bfloat16


def _split_matmul_waits(nc):
    """Walrus instruction structs have a single sem-wait slot (EventSemaphore
    has two). Peel excess waits onto NoOps (plain wait instructions on the
    same engine) inserted just before the overloaded instruction."""
    n = 0
    for bb in nc.m.functions[0].blocks:
        new_insts = []
        for inst in bb.instructions:
            cap = 2 if isinstance(inst, mybir.InstEventSemaphore) else 1
            if (
                inst.sync_info
                and inst.sync_info.on_wait
                and len(inst.sync_info.on_wait) > cap
            ):
                waits = list(inst.sync_info.on_wait)
                for w in waits[cap:]:
                    n += 1
                    nop = mybir.InstNoOp(name=f"I-xwait-{n}", ins=[], outs=[])
                    nop.engine = inst.engine
                    nop.sync_info = mybir.SyncInfo(on_wait=[w], on_update=[])
                    new_insts.append(nop)
                inst.sync_info.on_wait = waits[:cap]
            new_insts.append(inst)
        bb.instructions[:] = new_insts
    return n


def build_nc(qk_fp8=True, split_waits=True):
    qk_dt = FP8 if qk_fp8 else BF16

    nc = bass.Bass()
    # Host pre-swizzles inputs to SBUF-matching layouts so every DMA is one
    # long contiguous run per partition (8KB for x blocks, 2-4KB for weights).
    xT_d = nc.declare_dram_parameter("xT", [P, NSB, FO, SB], BF16, isOutput=False)
    w_d = {
        k: nc.declare_dram_parameter(k, [P, UO, FO, P], BF16, isOutput=False)
        for k in ("Wq", "Wk")
    }
    w_d["Wv"] = nc.declare_dram_parameter("Wv", [P, FO, U], BF16, isOutput=False)
    out_d = nc.declare_dram_parameter("out", [S, U], BF16, isOutput=True)

    TANH = mybir.ActivationFunctionType.Tanh
    EXP = mybir.ActivationFunctionType.Exp

    with tile.TileContext(nc) as tc:
        with (
            tc.tile_pool(name="wpool", bufs=1) as wpool,
            tc.tile_pool(name="qkv", bufs=1) as qkv,
            tc.tile_pool(name="smalls", bufs=1) as smalls,
            tc.tile_pool(name="recs", bufs=2) as recs,
            tc.tile_pool(name="evac", bufs=6) as evac,
            tc.tile_pool(name="exps", bufs=4) as exps,
            tc.tile_pool(name="ps_big", bufs=3, space="PSUM") as ps_big,
            tc.tile_pool(name="ps_v", bufs=2, space="PSUM") as ps_v,
            tc.tile_pool(name="ps_o", bufs=3, space="PSUM") as ps_o,
        ):
            # ---- phase 1: loads + projections. xT lives only here; its
            # SBUF space is released to the exp tiles afterwards. ----
            with tc.tile_pool(name="xpool", bufs=1) as xpool:
                # All DMAs ride the sync/SP queue: SP-issued DMAs fan out
                # over many SDMA engines, while scalar/gpsimd-issued DMAs
                # serialize on one engine (~3x slower — measured). Wq and
                # x-block 0 go first; block 0 is further split per fo chunk
                # so the first QT matmul starts as soon as possible.
                xT = xpool.tile([P, NSB, FO, SB], BF16)
                w_t = {
                    "Wq": wpool.tile([P, UO, FO, P], BF16, tag="Wq", name="w_Wq"),
                    "Wk": wpool.tile([P, UO, FO, P], BF16, tag="Wk", name="w_Wk"),
                    "Wv": wpool.tile([P, FO, U], BF16, tag="Wv", name="w_Wv"),
                }

                def dma_w(k, uo=None):
                    if uo is None:
                        nc.sync.dma_start(w_t[k][:], w_d[k][:])
                    else:
                        nc.sync.dma_start(w_t[k][:, uo], w_d[k][:, uo])

                def dma_x(sb, split=False):
                    if split:
                        for fo in range(FO):
                            nc.sync.dma_start(xT[:, sb, fo, :], xT_d[:, sb, fo, :])
                    else:
                        nc.sync.dma_start(xT[:, sb, :, :], xT_d[:, sb, :, :])

                # Byte-ordered so each consumer's data lands just in time:
                # Wq half 0 + x0-chunk 0 feed the first QT group; Wq half 1
                # right after chunk 0 (QT-uo1 re-reads resident chunks, so it
                # must not queue behind the whole x0 stream); Wk halves before
                # KT of block 0; Wv before V of block 0.
                dma_w("Wq", 0)
                for fo in range(FO):
                    nc.sync.dma_start(xT[:, 0, fo, :], xT_d[:, 0, fo, :])
                    if fo == 0:
                        dma_w("Wq", 1)
                dma_w("Wk", 0)
                dma_w("Wk", 1)
                dma_w("Wv")
                for sb in range(1, NSB):
                    dma_x(sb)

                # PE warmup: junk matmuls on a zeroed tile keep the PE busy
                # while the x DMAs land, so HAM un-throttles before real work.
                # gpsimd memsets the tile (bf16 bit pattern packed directly):
                # it is free ~1.5us before Vector at startup.
                warm = smalls.tile([P, SB], BF16, tag="warm")
                nc.gpsimd.memset(warm[:], 0.0)
                ps_w = ps_v.tile([P, SB], F32, tag="ps_v", name="ps_w")
                for _ in range(11):
                    nc.tensor.matmul(
                        ps_w[:], warm[:, :P], warm[:], start=True, stop=True
                    )

                # ---- projections (per s-block so PE starts as DMA lands) ----
                qT = qkv.tile([P, UO, S], qk_dt, tag="qT")
                kT = qkv.tile([P, UO, S], qk_dt, tag="kT")
                vv = qkv.tile([P, SO, VW], BF16, tag="vv")
                nc.gpsimd.memset(vv[:, :, U:VW], 1.0)

                ex_tiles = [None] * NQB
                for sb in range(NSB):
                    sl = slice(sb * SB, (sb + 1) * SB)
                    for wname, dst in (("Wq", qT), ("Wk", kT)):
                        for uo in range(UO):
                            ps = ps_big.tile([P, SB], F32, tag="ps_big")
                            for fo in range(FO):
                                nc.tensor.matmul(
                                    ps[:],
                                    w_t[wname][:, uo, fo, :],
                                    xT[:, sb, fo, :],
                                    start=(fo == 0),
                                    stop=(fo == FO - 1),
                                )
                                if sb == 0 and wname == "Wq" and uo == 0 and fo:
                                    # junk filler between the chunk-paced
                                    # first group's matmuls: absorbs x0 DMA
                                    # jitter without idling the PE (an idle
                                    # PE also loses its clock ramp)
                                    nc.tensor.matmul(
                                        ps_w[:, :U],
                                        warm[:, :P],
                                        warm[:, :U],
                                        start=True,
                                        stop=True,
                                    )
                            nc.scalar.activation(dst[:, uo, sl], ps[:], TANH)
                    for so in range(sb * SB // P, (sb + 1) * SB // P):
                        si = (so % (SB // P)) * P
                        ps = ps_v.tile([P, U], F32, tag="ps_v")
                        for fo in range(FO):
                            nc.tensor.matmul(
                                ps[:],
                                xT[:, sb, fo, si : si + P],
                                w_t["Wv"][:, fo, :],
                                start=(fo == 0),
                                stop=(fo == FO - 1),
                            )
                        nc.scalar.activation(vv[:, so, :U], ps[:], TANH)
                    # scores for the t-chunks this block's K just produced:
                    # fills PE gaps while the next x block's DMA lands. The
                    # needed qT q-slices come from blocks <= sb, available
                    # for qb <= sb; later qb wait for their qT (handled by
                    # Tile deps, but emitted only when ready to avoid stalls).
                    for qb in range(NQB):
                        if ex_tiles[qb] is None:
                            ex_tiles[qb] = exps.tile(
                                [P, SO, QB], BF16, tag="ex", name=f"ex{qb}"
                            )
                        if qb > sb:
                            continue
                        qsl = slice(qb * QB, (qb + 1) * QB)
                        for to in range(sb * (SO // NSB), (sb + 1) * (SO // NSB)):
                            ps = ps_big.tile([P, QB], F32, tag="ps_big")
                            if qk_fp8:
                                nc.tensor.matmul(
                                    ps[:],
                                    kT[:, :, to * P : (to + 1) * P],
                                    qT[:, :, qsl],
                                    start=True,
                                    stop=True,
                                    perf_mode=DR,
                                )
                            else:
                                for uo in range(UO):
                                    nc.tensor.matmul(
                                        ps[:],
                                        kT[:, uo, to * P : (to + 1) * P],
                                        qT[:, uo, qsl],
                                        start=(uo == 0),
                                        stop=(uo == UO - 1),
                                    )
                            nc.scalar.activation(
                                ex_tiles[qb][:, to, :], ps[:], EXP, scale=SCALE
                            )

            # ---- phase 2: remaining scores + output per query block.
            # Block qb's leftover scores (t-tiles from earlier s-blocks,
            # to < 4*qb) are emitted interleaved into block qb-1's output
            # groups, so their exp evacuations run on the Scalar engine
            # while the PE chews the previous block's out-matmuls. Each
            # out-group accumulates its freshest t-tiles LAST. ----
            def emit_score2(qb, to):
                ps = ps_big.tile([P, QB], F32, tag="ps_big")
                qsl = slice(qb * QB, (qb + 1) * QB)
                if qk_fp8:
                    nc.tensor.matmul(
                        ps[:],
                        kT[:, :, to * P : (to + 1) * P],
                        qT[:, :, qsl],
                        start=True,
                        stop=True,
                        perf_mode=DR,
                    )
                else:
                    for uo in range(UO):
                        nc.tensor.matmul(
                            ps[:],
                            kT[:, uo, to * P : (to + 1) * P],
                            qT[:, uo, qsl],
                            start=(uo == 0),
                            stop=(uo == UO - 1),
                        )
                nc.scalar.activation(
                    ex_tiles[qb][:, to, :], ps[:], EXP, scale=SCALE
                )

            pend = {
                qb: list(range(qb * (SO // NSB))) for qb in range(NQB)
            }
            for qb in range(NQB):
                ex = ex_tiles[qb]
                nxt = pend.get(qb + 1, [])
                per = (len(nxt) + (QB // P) - 1) // (QB // P) if nxt else 0
                for ss in range(QB // P):
                    for to in nxt[ss * per : (ss + 1) * per]:
                        emit_score2(qb + 1, to)
                    s0 = qb * QB + ss * P
                    ps = ps_o.tile([P, VW], F32, tag="ps_o")
                    to_order = list(range(qb * (SO // NSB), SO)) + list(
                        range(qb * (SO // NSB))
                    )
                    for n, to in enumerate(to_order):
                        nc.tensor.matmul(
                            ps[:],
                            ex[:, to, ss * P : (ss + 1) * P],
                            vv[:, to, :],
                            start=(n == 0),
                            stop=(n == SO - 1),
                        )
                    rec = recs.tile([P, 1], F32, tag="rec")
                    nc.vector.reciprocal(rec[:], ps[:, U : U + 1])
                    ot = evac.tile([P, U], BF16, tag="ot")
                    nc.vector.tensor_scalar_mul(ot[:], ps[:, :U], rec[:])
                    nc.sync.dma_start(out_d[s0 : s0 + P, :], ot[:])

    if split_waits:
        _split_matmul_waits(nc)
    return nc


_NC_CACHE = {}


def _get_nc(key=True):
    if key not in _NC_CACHE:
        _NC_CACHE[key] = build_nc(qk_fp8=key)
    return _NC_CACHE[key]


def _swizzle_w(w):
    # [F, U] -> [fi, fo, u]: contiguous 4KB per partition row.
    w = np.asarray(w, dtype=np.float32)
    return np.ascontiguousarray(
        w.reshape(FO, P, U).transpose(1, 0, 2).astype(NP_BF16)
    )


def _swizzle_w_halves(w):
    # [F, U] -> [fi, uo, fo, ui]: each uo half is one contiguous 2KB run
    # per partition, so it can be DMA'd independently.
    w = np.asarray(w, dtype=np.float32)
    return np.ascontiguousarray(
        w.reshape(FO, P, UO, P).transpose(1, 2, 0, 3).astype(NP_BF16)
    )


def _swizzle_x(xb):
    # [S, F] -> xT [fi, sb, fo, s]: each s-block DMA is one contiguous 8KB
    # run per partition.
    xT = np.asarray(xb, dtype=np.float32).T  # [F, S]
    return np.ascontiguousarray(
        xT.reshape(FO, P, NSB, SB).transpose(1, 2, 0, 3).astype(NP_BF16)
    )


def make_in_maps(x, Wq, Wk, Wv):
    Wq, Wk = _swizzle_w_halves(Wq), _swizzle_w_halves(Wk)
    Wv = _swizzle_w(Wv)
    return [
        {"xT": _swizzle_x(x[b]), "Wq": Wq, "Wk": Wk, "Wv": Wv}
        for b in range(B)
    ]


def kernel(x, Wq, Wk, Wv):
    nc = _get_nc()
    in_maps = make_in_maps(x, Wq, Wk, Wv)
    res = run_bass_kernel_spmd(nc, in_maps, core_ids=list(range(B)))
    return np.stack(
        [np.asarray(res.results[i]["out"], dtype=np.float32) for i in range(B)],
        axis=0,
    )


# revision 30
# speedup vs baseline: 1.0080x; 1.0029x over previous
"""Trainium2 Bass kernel for nn_AttentionTanh (B=8, S=2048, F=1024, U=256).

Data-parallel over batch: each of the 8 NeuronCores computes the full
attention for one batch example. No collectives.

Per-core dataflow (all matmuls via TensorE, out = lhsT.T @ rhs):
  xT   [F, S]  (host-swizzled bf16 input shard, F on partitions)
  QT   [u, s] = tanh(Wq.T @ x.T)  -> matmul(lhsT=Wq[f,u], rhs=xT[f,s])
  KT   [u, s] = tanh(Wk.T @ x.T)      QT/KT stored fp8e4 (scores run in
                                      fp8 DoubleRow; tanh bounds |q|<=1)
  V    [s, u] = tanh(x @ Wv)      -> matmul(lhsT=xT[f,s], rhs=Wv[f,u])
                V gets two fused ones-columns so the out-matmul also
                produces the softmax denominator (cols U:U+2).
  eST  [t, q] = exp(scale * K.T q) -> ONE fp8 DoubleRow matmul per
                (t-tile, q-block): contracts the full U=256 across the
                two uo planes of kT/qT at 2 rows/cycle.
                (tanh bounds scores to [-8, 8]; no max subtraction)
  out  [q, u] = (eST.T @ [V | 1 1]) row-normalized by column U (bf16).

Inputs are cast to bf16 on the host: halves HBM traffic (x: 8MB->4MB
per core) and the projection matmuls get Fast Weight Load.
"""

import os
import sys

import numpy as np
import ml_dtypes

for _p in ("/opt/trn_rl_repo", "/root/.axon_site/_ro/trn_rl_repo"):
    if os.path.isdir(_p) and _p not in sys.path:
        sys.path.append(_p)

import concourse.bass as bass
import concourse.mybir as mybir
import concourse.tile as tile
from concourse.bass_utils import run_bass_kernel_spmd

P = 128
B, S, F, U = 8, 2048, 1024, 256
FO, SO, UO = F // P, S // P, U // P  # 8, 16, 2
SB = 512                             # s-block width for DMA/projections
NSB = S // SB                        # 4
QB = 512                             # query-block width (free dim of eST)
NQB = S // QB                        # 4
SCALE = 1.0 / float(np.sqrt(F))      # 1/32
VW = U + 2                           # V plus fused ones columns
F32 = mybir.dt.float32
BF16 = mybir.dt.bfloat16
FP8 = mybir.dt.float8e4
DR = mybir.MatmulPerfMode.DoubleRow

NP_BF16 = ml_dtypes.bfloat16


def _split_matmul_waits(nc):
    """Walrus instruction structs have a single sem-wait slot (EventSemaphore
    has two). Peel excess waits onto NoOps (plain wait instructions on the
    same engine) inserted just before the overloaded instruction."""
    n = 0
    for bb in nc.m.functions[0].blocks:
        new_insts = []
        for inst in bb.instructions:
            cap = 2 if isinstance(inst, mybir.InstEventSemaphore) else 1
            if (
                inst.sync_info
                and inst.sync_info.on_wait
                and len(inst.sync_info.on_wait) > cap
            ):
                waits = list(inst.sync_info.on_wait)
                for w in waits[cap:]:
                    n += 1
                    nop = mybir.InstNoOp(name=f"I-xwait-{n}", ins=[], outs=[])
                    nop.engine = inst.engine
                    nop.sync_info = mybir.SyncInfo(on_wait=[w], on_update=[])
                    new_insts.append(nop)
                inst.sync_info.on_wait = waits[:cap]
            new_insts.append(inst)
        bb.instructions[:] = new_insts
    return n


def build_nc(qk_fp8=True, split_waits=True):
    qk_dt = FP8 if qk_fp8 else BF16

    nc = bass.Bass()
    # Host pre-swizzles inputs to SBUF-matching layouts so every DMA is one
    # long contiguous run per partition (8KB for x blocks, 2-4KB for weights).
    xT_d = nc.declare_dram_parameter("xT", [P, NSB, FO, SB], BF16, isOutput=False)
    w_d = {
        k: nc.declare_dram_parameter(k, [P, UO, FO, P], BF16, isOutput=False)
        for k in ("Wq", "Wk")
    }
    w_d["Wv"] = nc.declare_dram_parameter("Wv", [P, FO, U], BF16, isOutput=False)
    out_d = nc.declare_dram_parameter("out", [S, U], BF16, isOutput=True)

    TANH = mybir.ActivationFunctionType.Tanh
    EXP = mybir.ActivationFunctionType.Exp

    with tile.TileContext(nc) as tc:
        with (
            tc.tile_pool(name="wpool", bufs=1) as wpool,
            tc.tile_pool(name="qkv", bufs=1) as qkv,
            tc.tile_pool(name="smalls", bufs=1) as smalls,
            tc.tile_pool(name="recs", bufs=2) as recs,
            tc.tile_pool(name="evac", bufs=6) as evac,
            tc.tile_pool(name="exps", bufs=4) as exps,
            tc.tile_pool(name="ps_big", bufs=3, space="PSUM") as ps_big,
            tc.tile_pool(name="ps_v", bufs=2, space="PSUM") as ps_v,
            tc.tile_pool(name="ps_o", bufs=3, space="PSUM") as ps_o,
        ):
            # ---- phase 1: loads + projections. xT lives only here; its
            # SBUF space is released to the exp tiles afterwards. ----
            with tc.tile_pool(name="xpool", bufs=1) as xpool:
                # All DMAs ride the sync/SP queue: SP-issued DMAs fan out
                # over many SDMA engines, while scalar/gpsimd-issued DMAs
                # serialize on one engine (~3x slower — measured). Wq and
                # x-block 0 go first; block 0 is further split per fo chunk
                # so the first QT matmul starts as soon as possible.
                xT = xpool.tile([P, NSB, FO, SB], BF16)
                w_t = {
                    "Wq": wpool.tile([P, UO, FO, P], BF16, tag="Wq", name="w_Wq"),
                    "Wk": wpool.tile([P, UO, FO, P], BF16, tag="Wk", name="w_Wk"),
                    "Wv": wpool.tile([P, FO, U], BF16, tag="Wv", name="w_Wv"),
                }

                def dma_w(k, uo=None):
                    if uo is None:
                        nc.sync.dma_start(w_t[k][:], w_d[k][:])
                    else:
                        nc.sync.dma_start(w_t[k][:, uo], w_d[k][:, uo])

                def dma_x(sb, split=False):
                    if split:
                        for fo in range(FO):
                            nc.sync.dma_start(xT[:, sb, fo, :], xT_d[:, sb, fo, :])
                    else:
                        nc.sync.dma_start(xT[:, sb, :, :], xT_d[:, sb, :, :])

                # Byte-ordered so each consumer's data lands just in time:
                # Wq half 0 + x0-chunk 0 feed the first QT group; Wq half 1
                # right after chunk 0 (QT-uo1 re-reads resident chunks, so it
                # must not queue behind the whole x0 stream); Wk halves before
                # KT of block 0; Wv before V of block 0.
                dma_w("Wq", 0)
                for fo in range(FO):
                    nc.sync.dma_start(xT[:, 0, fo, :], xT_d[:, 0, fo, :])
                    if fo == 0:
                        dma_w("Wq", 1)
                dma_w("Wk", 0)
                dma_w("Wk", 1)
                dma_w("Wv")
                for sb in range(1, NSB):
                    dma_x(sb)

                # PE warmup: junk matmuls on a zeroed tile keep the PE busy
                # while the x DMAs land, so HAM un-throttles before real work.
                # gpsimd memsets the tile (bf16 bit pattern packed directly):
                # it is free ~1.5us before Vector at startup.
                warm = smalls.tile([P, SB], BF16, tag="warm")
                nc.gpsimd.memset(warm[:], 0.0)
                ps_w = ps_v.tile([P, SB], F32, tag="ps_v", name="ps_w")
                for _ in range(8):
                    nc.tensor.matmul(
                        ps_w[:], warm[:, :P], warm[:], start=True, stop=True
                    )

                # ---- projections (per s-block so PE starts as DMA lands) ----
                qT = qkv.tile([P, UO, S], qk_dt, tag="qT")
                kT = qkv.tile([P, UO, S], qk_dt, tag="kT")
                vv = qkv.tile([P, SO, VW], BF16, tag="vv")
                nc.gpsimd.memset(vv[:, :, U:VW], 1.0)

                ex_tiles = [None] * NQB
                for sb in range(NSB):
                    sl = slice(sb * SB, (sb + 1) * SB)
                    for wname, dst in (("Wq", qT), ("Wk", kT)):
                        for uo in range(UO):
                            ps = ps_big.tile([P, SB], F32, tag="ps_big")
                            for fo in range(FO):
                                nc.tensor.matmul(
                                    ps[:],
                                    w_t[wname][:, uo, fo, :],
                                    xT[:, sb, fo, :],
                                    start=(fo == 0),
                                    stop=(fo == FO - 1),
                                )
                                if sb == 0 and wname == "Wq" and uo == 0 and fo:
                                    # junk filler between the chunk-paced
                                    # first group's matmuls: absorbs x0 DMA
                                    # jitter without idling the PE (an idle
                                    # PE also loses its clock ramp)
                                    nc.tensor.matmul(
                                        ps_w[:, :U],
                                        warm[:, :P],
                                        warm[:, :U],
                                        start=True,
                                        stop=True,
                                    )
                            nc.scalar.activation(dst[:, uo, sl], ps[:], TANH)
                    for so in range(sb * SB // P, (sb + 1) * SB // P):
                        si = (so % (SB // P)) * P
                        ps = ps_v.tile([P, U], F32, tag="ps_v")
                        for fo in range(FO):
                            nc.tensor.matmul(
                                ps[:],
                                xT[:, sb, fo, si : si + P],
                                w_t["Wv"][:, fo, :],
                                start=(fo == 0),
                                stop=(fo == FO - 1),
                            )
                        nc.scalar.activation(vv[:, so, :U], ps[:], TANH)
                    # scores for the t-chunks this block's K just produced:
                    # fills PE gaps while the next x block's DMA lands. The
                    # needed qT q-slices come from blocks <= sb, available
                    # for qb <= sb; later qb wait for their qT (handled by
                    # Tile deps, but emitted only when ready to avoid stalls).
                    for qb in range(NQB):
                        if ex_tiles[qb] is None:
                            ex_tiles[qb] = exps.tile(
                                [P, SO, QB], BF16, tag="ex", name=f"ex{qb}"
                            )
                        if qb > sb:
                            continue
                        qsl = slice(qb * QB, (qb + 1) * QB)
                        for to in range(sb * (SO // NSB), (sb + 1) * (SO // NSB)):
                            ps = ps_big.tile([P, QB], F32, tag="ps_big")
                            if qk_fp8:
                                nc.tensor.matmul(
                                    ps[:],
                                    kT[:, :, to * P : (to + 1) * P],
                                    qT[:, :, qsl],
                                    start=True,
                                    stop=True,
                                    perf_mode=DR,
                                )
                            else:
                                for uo in range(UO):
                                    nc.tensor.matmul(
                                        ps[:],
                                        kT[:, uo, to * P : (to + 1) * P],
                                        qT[:, uo, qsl],
                                        start=(uo == 0),
                                        stop=(uo == UO - 1),
                                    )
                            nc.scalar.activation(
                                ex_tiles[qb][:, to, :], ps[:], EXP, scale=SCALE
                            )

            # ---- phase 2: remaining scores + output per query block.
            # Block qb's leftover scores (t-tiles from earlier s-blocks,
            # to < 4*qb) are emitted interleaved into block qb-1's output
            # groups, so their exp evacuations run on the Scalar engine
            # while the PE chews the previous block's out-matmuls. Each
            # out-group accumulates its freshest t-tiles LAST. ----
            def emit_score2(qb, to):
                ps = ps_big.tile([P, QB], F32, tag="ps_big")
                qsl = slice(qb * QB, (qb + 1) * QB)
                if qk_fp8:
                    nc.tensor.matmul(
                        ps[:],
                        kT[:, :, to * P : (to + 1) * P],
                        qT[:, :, qsl],
                        start=True,
                        stop=True,
                        perf_mode=DR,
                    )
                else:
                    for uo in range(UO):
                        nc.tensor.matmul(
                            ps[:],
                            kT[:, uo, to * P : (to + 1) * P],
                            qT[:, uo, qsl],
                            start=(uo == 0),
                            stop=(uo == UO - 1),
                        )
                nc.scalar.activation(
                    ex_tiles[qb][:, to, :], ps[:], EXP, scale=SCALE
                )

            pend = {
                qb: list(range(qb * (SO // NSB))) for qb in range(NQB)
            }
            for qb in range(NQB):
                ex = ex_tiles[qb]
                nxt = pend.get(qb + 1, [])
                per = (len(nxt) + (QB // P) - 1) // (QB // P) if nxt else 0
                for ss in range(QB // P):
                    for to in nxt[ss * per : (ss + 1) * per]:
                        emit_score2(qb + 1, to)
                    s0 = qb * QB + ss * P
                    ps = ps_o.tile([P, VW], F32, tag="ps_o")
                    to_order = list(range(qb * (SO // NSB), SO)) + list(
                        range(qb * (SO // NSB))
                    )
                    for n, to in enumerate(to_order):
                        nc.tensor.matmul(
                            ps[:],
                            ex[:, to, ss * P : (ss + 1) * P],
                            vv[:, to, :],
                            start=(n == 0),
                            stop=(n == SO - 1),
                        )
                    rec = recs.tile([P, 1], F32, tag="rec")
                    nc.vector.reciprocal(rec[:], ps[:, U : U + 1])
                    ot = evac.tile([P, U], BF16, tag="ot")
                    nc.vector.tensor_scalar_mul(ot[:], ps[:, :U], rec[:])
                    nc.sync.dma_start(out_d[s0 : s0 + P, :], ot[:])

    if split_waits:
        _split_matmul_waits(nc)
    return nc


_NC_CACHE = {}


def _get_nc(key=True):
    if key not in _NC_CACHE:
        _NC_CACHE[key] = build_nc(qk_fp8=key)
    return _NC_CACHE[key]


def _swizzle_w(w):
    # [F, U] -> [fi, fo, u]: contiguous 4KB per partition row.
    w = np.asarray(w, dtype=np.float32)
    return np.ascontiguousarray(
        w.reshape(FO, P, U).transpose(1, 0, 2).astype(NP_BF16)
    )


def _swizzle_w_halves(w):
    # [F, U] -> [fi, uo, fo, ui]: each uo half is one contiguous 2KB run
    # per partition, so it can be DMA'd independently.
    w = np.asarray(w, dtype=np.float32)
    return np.ascontiguousarray(
        w.reshape(FO, P, UO, P).transpose(1, 2, 0, 3).astype(NP_BF16)
    )


def _swizzle_x(xb):
    # [S, F] -> xT [fi, sb, fo, s]: each s-block DMA is one contiguous 8KB
    # run per partition.
    xT = np.asarray(xb, dtype=np.float32).T  # [F, S]
    return np.ascontiguousarray(
        xT.reshape(FO, P, NSB, SB).transpose(1, 2, 0, 3).astype(NP_BF16)
    )


def make_in_maps(x, Wq, Wk, Wv):
    Wq, Wk = _swizzle_w_halves(Wq), _swizzle_w_halves(Wk)
    Wv = _swizzle_w(Wv)
    return [
        {"xT": _swizzle_x(x[b]), "Wq": Wq, "Wk": Wk, "Wv": Wv}
        for b in range(B)
    ]


def kernel(x, Wq, Wk, Wv):
    nc = _get_nc()
    in_maps = make_in_maps(x, Wq, Wk, Wv)
    res = run_bass_kernel_spmd(nc, in_maps, core_ids=list(range(B)))
    return np.stack(
        [np.asarray(res.results[i]["out"], dtype=np.float32) for i in range(B)],
        axis=0,
    )
